# revision 29
# baseline (speedup 1.0000x reference)
"""TRN2 Bass kernel for nn_SynthesisLayer (StyleGAN-style modulated 3D conv).

Math: for each sample b
  styles = w[b] @ affine_weight.T / sqrt(512) + affine_bias          [Cin]
  wmod   = weight * styles[None,:,None]                              [Co,Ci,27]
  dcoef  = rsqrt(sum_{ci,k} wmod^2 + 1e-8)                           [Co]
  y      = dcoef * conv3d(x[b], wmod, pad=1) + noise_const*ns + bias
  out    = clip(lrelu(y)*sqrt(2), -256, 256)

Device implementation (per core):
  - styles/modulation/demodulation are folded on the host: the device
    receives pre-modulated, pre-demodulated bf16 weights, so the conv is
    27 shifted bf16 matmuls (contraction over Cin=128 on partitions)
    accumulated in PSUM, then scale+noise+prelu+clamp on DVE/ACT.
  - Sharding: 8 cores = 4 samples x 2 D-halves. The upper D-half runs
    D-REVERSED (reversed input slices, kd-reversed weights, output
    un-reversed on host) so that for EVERY core the first output slice
    is a volume boundary: its 9 kd=-1 taps are identically zero and are
    skipped. Likewise output rows 0/31 are split into 1-row sub-chunks
    that skip the zero-row kh taps (d>=1 only: the d=0 instruction
    stream is kept in the exact shape that starts the PE past its
    p-state ramp window). Together ~4% of PE work is skipped.
  - x slab (17 slices + guards, whole D-half) is SBUF-resident in bf16
    and DMA-streamed slice-by-slice so matmuls chase the DMA front;
    weights/noise also ship bf16 (halves DMA); PSUM stays f32.
  - The final chunk is split into small sub-chunks with an all-DVE
    post chain and a merged last output DMA to shorten the drain tail.
"""

import math
import os
import sys

for _p in ("/opt/trn_rl_repo", "/root/.axon_site/_ro/trn_rl_repo"):
    if os.path.isdir(_p) and _p not in sys.path:
        sys.path.insert(0, _p)

import numpy as np
import ml_dtypes

import concourse.mybir as mybir
from concourse import bacc
from concourse.tile import TileContext
from concourse.bass_utils import run_bass_kernel_spmd

BF16 = ml_dtypes.bfloat16

P = 128          # Cin = Cout = 128
RES = 32
B = 4
W_DIM = 512
ROW = 33                   # padded row width  (32 real + 1 zero)
SLICE = ROW * ROW          # 1089: padded slice (32 real rows + 1 zero row)
LEAD = 34                  # leading zero guard (one row + one elem)
NSLICES = 17               # 16 output slices + 1 far-side halo
DHALF = 16                 # output D slices per core
NOUT = DHALF * RES * RES   # 16384
NSLAB = LEAD + NSLICES * SLICE + 46   # 18593; max AP end = 18548
NCHUNK = 512               # psum tile free size (one PSUM bank of fp32)
LRELU_ALPHA = 0.2
LRELU_GAIN = math.sqrt(2.0)
CLAMP = 256.0

f32 = mybir.dt.float32
bf16 = mybir.dt.bfloat16
AF = mybir.ActivationFunctionType

_NC_CACHE = None
LAST_EXEC_NS = None

# tap index in the weight tensor: k = (kd+1)*9 + (kh+1)*3 + (kw+1)
def _tap_idx(kd, kh, kw):
    return (kd + 1) * 9 + (kh + 1) * 3 + (kw + 1)


def build_nc():
    nc = bacc.Bacc("TRN2", target_bir_lowering=False, debug=False, num_devices=8)

    xs = nc.dram_tensor("xs", [P, NSLAB], bf16, kind="ExternalInput")
    wt = nc.dram_tensor("wt", [P, 27, P], bf16, kind="ExternalInput")
    nz = nc.dram_tensor("nz", [1, NOUT], bf16, kind="ExternalInput")
    bb = nc.dram_tensor("bb", [P, 3], f32, kind="ExternalInput")  # bias', lo, hi
    y = nc.dram_tensor("y", [P, NOUT], f32, kind="ExternalOutput")

    with TileContext(nc) as tc:
        with (
            tc.tile_pool(name="slab", bufs=1) as slabp,
            tc.tile_pool(name="nzp", bufs=4) as nzp,
            tc.tile_pool(name="outp", bufs=4) as outp,
            tc.tile_pool(name="cpsum", bufs=8, space="PSUM") as cpsum,
        ):
            xs_sb = slabp.tile([P, NSLAB], bf16)
            wt_sb = slabp.tile([P, 27, P], bf16)
            bb_sb = slabp.tile([P, 3], f32)

            def dma_slice(j):
                """DMA padded slice j (first includes LEAD, last the tail)."""
                lo = 0 if j == 0 else LEAD + j * SLICE
                hi = NSLAB if j == NSLICES - 1 else LEAD + (j + 1) * SLICE
                nc.sync.dma_start(xs_sb[:, lo:hi], xs[:, lo:hi])

            # startup order: tiny first pieces so the first matmul's inputs
            # land as early as possible (descriptor issue is ~650ns each)
            nc.sync.dma_start(wt_sb[:, 9:12, :], wt[:, 9:12, :])   # kd=0,kh=-1
            nc.sync.dma_start(xs_sb[:, 0:600], xs[:, 0:600])       # s0 rows<=16
            nc.sync.dma_start(wt_sb[:, 12:18, :], wt[:, 12:18, :])  # kd=0 rest
            nc.sync.dma_start(xs_sb[:, 600 : LEAD + SLICE],
                              xs[:, 600 : LEAD + SLICE])            # s0 rest
            nc.sync.dma_start(wt_sb[:, 18:27, :], wt[:, 18:27, :])  # kd=+1
            dma_slice(1)
            nc.sync.dma_start(wt_sb[:, 0:9, :], wt[:, 0:9, :])      # kd=-1
            nc.sync.dma_start(bb_sb[:], bb[:])
            dma_slice(2)
            dma_slice(3)

            next_slice = 4

            for d in range(DHALF):
                if next_slice < NSLICES:
                    dma_slice(next_slice)
                    next_slice += 1
                kds = (0, 1) if d == 0 else (-1, 0, 1)
                KH_ALL = (-1, 0, 1)
                for h in range(2):
                    # 1-row boundary subs (d>=2 only: the d=0/1 stream is
                    # kept in the exact shape that keeps the PE p-state warm)
                    # skip the zero-row kh taps: row 0 has no kh=-1
                    # contribution, row 31 no kh=+1. The final chunk is
                    # split further so the drain tail is short.
                    last = d == DHALF - 1 and h == 1
                    if d < 1:
                        row_ranges = [(0, 16, KH_ALL)]
                    elif h == 0:
                        row_ranges = [(0, 1, (0, 1)), (1, 16, KH_ALL)]
                    elif last:
                        row_ranges = [(0, 4, KH_ALL), (4, 8, KH_ALL),
                                      (8, 12, KH_ALL), (12, 15, KH_ALL),
                                      (15, 16, (-1, 0))]
                    else:
                        row_ranges = [(0, 15, KH_ALL), (15, 16, (-1, 0))]
                    off = d * 1024 + h * NCHUNK
                    nz_bc = nzp.tile([P, 1, NCHUNK], bf16, tag="nz")
                    nc.sync.dma_start(
                        nz_bc[:],
                        nz[:, off : off + NCHUNK].partition_broadcast(P),
                    )
                    ut = outp.tile([P, NCHUNK], f32, tag="out")
                    for r0, r1, khs in row_ranges:
                        nrows = r1 - r0
                        ncols = nrows * RES
                        c0 = r0 * RES
                        pt = cpsum.tile([P, ncols], f32, tag="conv")
                        n_mm = len(kds) * len(khs) * 3
                        i = 0
                        for kd in kds:
                            for kh in khs:
                                for kw in (-1, 0, 1):
                                    q = (LEAD + (d + kd) * SLICE
                                         + (16 * h + r0 + kh) * ROW + kw)
                                    wlen = nrows * ROW
                                    rhs = xs_sb[:, q : q + wlen].rearrange(
                                        "p (r c) -> p r c", c=ROW
                                    )[:, :, :RES]
                                    nc.tensor.matmul(
                                        pt[:], wt_sb[:, _tap_idx(kd, kh, kw), :],
                                        rhs,
                                        start=(i == 0), stop=(i == n_mm - 1),
                                    )
                                    i += 1
                        # ut = psum*sqrt(2) + noise_pre (pre-scaled on host)
                        uv = ut[:, c0 : c0 + ncols]
                        nc.vector.scalar_tensor_tensor(
                            uv, pt[:], LRELU_GAIN,
                            nz_bc[:, 0, c0 : c0 + ncols],
                            mybir.AluOpType.mult, mybir.AluOpType.add,
                        )
                        if last and r0 == 15:
                            # all-DVE post for the very last sub: no
                            # cross-engine hops on the drain path.
                            # lrelu(z) == max(z, 0.2*z)
                            nc.vector.tensor_scalar_add(uv, uv, bb_sb[:, 0:1])
                            nc.vector.scalar_tensor_tensor(
                                uv, uv, LRELU_ALPHA, uv,
                                mybir.AluOpType.mult, mybir.AluOpType.max,
                            )
                        else:
                            nc.scalar.activation(
                                uv, uv, AF.Prelu,
                                bias=bb_sb[:, 0:1], scale=1.0,
                                alpha=LRELU_ALPHA,
                            )
                        nc.vector.tensor_scalar(
                            uv, uv, CLAMP, -CLAMP,
                            mybir.AluOpType.min, mybir.AluOpType.max,
                        )
                        if last and r1 <= 12:
                            nc.sync.dma_start(
                                y[:, off + c0 : off + c0 + ncols],
                                ut[:, c0 : c0 + ncols],
                            )
                    if last:
                        # one DMA for the final two subs (rows 12..16)
                        nc.sync.dma_start(
                            y[:, off + 384 : off + NCHUNK], ut[:, 384:NCHUNK]
                        )
                    else:
                        nc.sync.dma_start(y[:, off : off + NCHUNK], ut[:])

    nc.compile()
    return nc


def _get_nc():
    global _NC_CACHE
    if _NC_CACHE is None:
        _NC_CACHE = build_nc()
    return _NC_CACHE


def _make_core_inputs(x, w, affine_weight, affine_bias, weight, noise_const,
                      noise_strength, bias):
    """Host-side prep: styles, modulation, demodulation, layout, bf16."""
    styles = (w @ affine_weight.T) / math.sqrt(W_DIM) + affine_bias      # [B,Cin]
    wmod = weight[None] * styles[:, None, :, None, None, None]           # [B,Co,Ci,3,3,3]
    dco = 1.0 / np.sqrt((wmod ** 2).sum(axis=(2, 3, 4, 5)) + 1e-8)       # [B,Co]
    wd = wmod * dco[:, :, None, None, None, None]                        # demodulated

    xb = x.astype(BF16)                                                  # [B,Ci,32,32,32]
    nzs = (noise_const * float(noise_strength.reshape(-1)[0])
           * LRELU_GAIN).astype(BF16)                                    # [32,32,32]
    bgain = bias * LRELU_GAIN
    bb_host = np.stack(
        [bgain,                                  # prelu bias
         -CLAMP / LRELU_ALPHA - bgain,           # pre-act clamp lo
         CLAMP - bgain],                         # pre-act clamp hi
        axis=1,
    ).astype(np.float32)                                                 # [P,3]

    in_maps = []
    for c in range(8):
        b, half = divmod(c, 2)
        # weights: [ci, kd, kh, kw, co]; kd reversed for the upper half
        arr = wd[b].transpose(1, 2, 3, 4, 0)
        if half == 1:
            arr = arr[:, ::-1]
        wt_host = np.ascontiguousarray(arr.reshape(P, 27, P)).astype(BF16)

        # slab: local slice j = global (j) or (31-j); 17 slices incl. halo
        if half == 0:
            src = xb[b, :, 0:NSLICES]                      # 0..16
            nz_loc = nzs[0:DHALF]
        else:
            src = xb[b, :, 31:31 - NSLICES:-1]             # 31..15
            nz_loc = nzs[31:31 - DHALF:-1]
        slab = np.zeros((P, NSLAB), BF16)
        view = slab[:, LEAD:LEAD + NSLICES * SLICE].reshape(P, NSLICES, ROW, ROW)
        view[:, :, :RES, :RES] = src
        nz_host = np.ascontiguousarray(nz_loc.reshape(1, NOUT))

        in_maps.append({
            "xs": slab,
            "wt": wt_host,
            "nz": nz_host,
            "bb": bb_host,
        })
    return in_maps


def kernel(x, w, affine_weight, affine_bias, weight, noise_const,
           noise_strength, bias):
    global LAST_EXEC_NS
    x = np.asarray(x, np.float32)
    w = np.asarray(w, np.float32)
    affine_weight = np.asarray(affine_weight, np.float32)
    affine_bias = np.asarray(affine_bias, np.float32)
    weight = np.asarray(weight, np.float32)
    noise_const = np.asarray(noise_const, np.float32)
    noise_strength = np.asarray(noise_strength, np.float32)
    bias = np.asarray(bias, np.float32)

    nc = _get_nc()
    in_maps = _make_core_inputs(
        x, w, affine_weight, affine_bias, weight, noise_const,
        noise_strength, bias,
    )
    trace = bool(os.environ.get("KERNEL_TRACE"))
    if trace:
        from concourse.bass_utils import axon_active

        if axon_active():
            try:  # axon NTFF capture needs the profile hook; absent in some pods
                from antenv.axon_hooks import get_axon_ntff_profile_hook  # noqa: F401
            except ImportError:
                trace = False
    res = run_bass_kernel_spmd(nc, in_maps, core_ids=list(range(8)), trace=trace)
    LAST_EXEC_NS = res.exec_time_ns

    out = np.empty((B, P, RES, RES, RES), np.float32)
    for c in range(8):
        b, half = divmod(c, 2)
        yc = res.results[c]["y"].reshape(P, DHALF, RES, RES)
        if half == 0:
            out[b, :, 0:DHALF] = yc
        else:
            out[b, :, 31:31 - DHALF:-1] = yc
    return out


# revision 31
# speedup vs baseline: 1.0172x; 1.0172x over previous
"""TRN2 Bass kernel for nn_SynthesisLayer (StyleGAN-style modulated 3D conv).

Math: for each sample b
  styles = w[b] @ affine_weight.T / sqrt(512) + affine_bias          [Cin]
  wmod   = weight * styles[None,:,None]                              [Co,Ci,27]
  dcoef  = rsqrt(sum_{ci,k} wmod^2 + 1e-8)                           [Co]
  y      = dcoef * conv3d(x[b], wmod, pad=1) + noise_const*ns + bias
  out    = clip(lrelu(y)*sqrt(2), -256, 256)

Device implementation (per core):
  - styles/modulation/demodulation are folded on the host: the device
    receives pre-modulated, pre-demodulated bf16 weights, so the conv is
    27 shifted bf16 matmuls (contraction over Cin=128 on partitions)
    accumulated in PSUM, then scale+noise+prelu+clamp on DVE/ACT.
  - Sharding: 8 cores = 4 samples x 2 D-halves. The upper D-half runs
    D-REVERSED (reversed input slices, kd-reversed weights, output
    un-reversed on host) so that for EVERY core the first output slice
    is a volume boundary: its 9 kd=-1 taps are identically zero and are
    skipped. Likewise output rows 0/31 are split into 1-row sub-chunks
    that skip the zero-row kh taps (d>=1 only: the d=0 instruction
    stream is kept in the exact shape that starts the PE past its
    p-state ramp window). Together ~4% of PE work is skipped.
  - x slab (17 slices + guards, whole D-half) is SBUF-resident in bf16
    and DMA-streamed slice-by-slice so matmuls chase the DMA front;
    weights/noise also ship bf16 (halves DMA); PSUM stays f32.
  - The final chunk is split into small sub-chunks with an all-DVE
    post chain and a merged last output DMA to shorten the drain tail.
"""

import math
import os
import sys

for _p in ("/opt/trn_rl_repo", "/root/.axon_site/_ro/trn_rl_repo"):
    if os.path.isdir(_p) and _p not in sys.path:
        sys.path.insert(0, _p)

import numpy as np
import ml_dtypes

import concourse.mybir as mybir
from concourse import bacc
from concourse.tile import TileContext
from concourse.bass_utils import run_bass_kernel_spmd

BF16 = ml_dtypes.bfloat16

P = 128          # Cin = Cout = 128
RES = 32
B = 4
W_DIM = 512
ROW = 33                   # padded row width  (32 real + 1 zero)
SLICE = ROW * ROW          # 1089: padded slice (32 real rows + 1 zero row)
LEAD = 34                  # leading zero guard (one row + one elem)
NSLICES = 17               # 16 output slices + 1 far-side halo
DHALF = 16                 # output D slices per core
NOUT = DHALF * RES * RES   # 16384
NSLAB = LEAD + NSLICES * SLICE + 46   # 18593; max AP end = 18548
NCHUNK = 512               # psum tile free size (one PSUM bank of fp32)
LRELU_ALPHA = 0.2
LRELU_GAIN = math.sqrt(2.0)
CLAMP = 256.0

f32 = mybir.dt.float32
bf16 = mybir.dt.bfloat16
AF = mybir.ActivationFunctionType

_NC_CACHE = None
LAST_EXEC_NS = None

# tap index in the weight tensor: k = (kd+1)*9 + (kh+1)*3 + (kw+1)
def _tap_idx(kd, kh, kw):
    return (kd + 1) * 9 + (kh + 1) * 3 + (kw + 1)


def build_nc():
    nc = bacc.Bacc("TRN2", target_bir_lowering=False, debug=False, num_devices=8)

    xs = nc.dram_tensor("xs", [P, NSLAB], bf16, kind="ExternalInput")
    wt = nc.dram_tensor("wt", [P, 27, P], bf16, kind="ExternalInput")
    nz = nc.dram_tensor("nz", [1, NOUT], bf16, kind="ExternalInput")
    bb = nc.dram_tensor("bb", [P, 3], f32, kind="ExternalInput")  # bias', lo, hi
    y = nc.dram_tensor("y", [P, NOUT], f32, kind="ExternalOutput")

    with TileContext(nc) as tc:
        with (
            tc.tile_pool(name="slab", bufs=1) as slabp,
            tc.tile_pool(name="nzp", bufs=4) as nzp,
            tc.tile_pool(name="outp", bufs=4) as outp,
            tc.tile_pool(name="cpsum", bufs=8, space="PSUM") as cpsum,
        ):
            xs_sb = slabp.tile([P, NSLAB], bf16)
            wt_sb = slabp.tile([P, 27, P], bf16)
            bb_sb = slabp.tile([P, 3], f32)

            def dma_slice(j):
                """DMA padded slice j (first includes LEAD, last the tail)."""
                lo = 0 if j == 0 else LEAD + j * SLICE
                hi = NSLAB if j == NSLICES - 1 else LEAD + (j + 1) * SLICE
                nc.sync.dma_start(xs_sb[:, lo:hi], xs[:, lo:hi])

            # startup order: tiny first pieces so the first matmul's inputs
            # land as early as possible (descriptor issue is ~650ns each)
            nc.sync.dma_start(wt_sb[:, 9:12, :], wt[:, 9:12, :])   # kd=0,kh=-1
            nc.sync.dma_start(xs_sb[:, 0:600], xs[:, 0:600])       # s0 rows<=16
            nc.sync.dma_start(wt_sb[:, 12:18, :], wt[:, 12:18, :])  # kd=0 rest
            nc.sync.dma_start(xs_sb[:, 600 : LEAD + SLICE],
                              xs[:, 600 : LEAD + SLICE])            # s0 rest
            nc.sync.dma_start(wt_sb[:, 18:27, :], wt[:, 18:27, :])  # kd=+1
            dma_slice(1)
            nc.sync.dma_start(wt_sb[:, 0:9, :], wt[:, 0:9, :])      # kd=-1
            nc.sync.dma_start(bb_sb[:], bb[:])
            dma_slice(2)
            dma_slice(3)

            next_slice = 4

            for d in range(DHALF):
                if next_slice < NSLICES:
                    dma_slice(next_slice)
                    next_slice += 1
                kds = (0, 1) if d == 0 else (-1, 0, 1)
                KH_ALL = (-1, 0, 1)
                KW_ALL = (-1, 0, 1)
                for h in range(2):
                    # boundary subs skip zero-guard taps: row 0 has no kh=-1
                    # contribution, row 31 no kh=+1; likewise col 0 skips
                    # kw=-1 and col 31 kw=+1 via 1-col subs. d=0 keeps the
                    # exact stream shape that keeps the PE p-state warm; the
                    # final chunk keeps row-splits so the drain tail is short.
                    last = d == DHALF - 1 and h == 1
                    if d < 1:
                        subs = [(0, 16, 0, 32, KH_ALL, KW_ALL)]
                    elif h == 0:
                        subs = [(0, 1, 0, 32, (0, 1), KW_ALL),
                                (1, 16, 0, 1, KH_ALL, (0, 1)),
                                (1, 16, 1, 31, KH_ALL, KW_ALL),
                                (1, 16, 31, 32, KH_ALL, (-1, 0))]
                    elif last:
                        subs = [(0, 4, 0, 32, KH_ALL, KW_ALL),
                                (4, 8, 0, 32, KH_ALL, KW_ALL),
                                (8, 12, 0, 32, KH_ALL, KW_ALL),
                                (12, 15, 0, 32, KH_ALL, KW_ALL),
                                (15, 16, 0, 32, (-1, 0), KW_ALL)]
                    else:
                        subs = [(0, 15, 0, 1, KH_ALL, (0, 1)),
                                (0, 15, 1, 31, KH_ALL, KW_ALL),
                                (0, 15, 31, 32, KH_ALL, (-1, 0)),
                                (15, 16, 0, 32, (-1, 0), KW_ALL)]
                    off = d * 1024 + h * NCHUNK
                    nz_bc = nzp.tile([P, 1, NCHUNK], bf16, tag="nz")
                    nc.sync.dma_start(
                        nz_bc[:],
                        nz[:, off : off + NCHUNK].partition_broadcast(P),
                    )
                    ut = outp.tile([P, NCHUNK], f32, tag="out")
                    ut3 = ut[:].rearrange("p (r c) -> p r c", c=RES)
                    nz3 = nz_bc[:, 0, :].rearrange("p (r c) -> p r c", c=RES)
                    for r0, r1, cw0, cw1, khs, kws in subs:
                        nrows = r1 - r0
                        ncw = cw1 - cw0
                        pt = cpsum.tile([P, nrows, ncw], f32, tag="conv")
                        n_mm = len(kds) * len(khs) * len(kws)
                        i = 0
                        for kd in kds:
                            for kh in khs:
                                for kw in kws:
                                    q = (LEAD + (d + kd) * SLICE
                                         + (16 * h + r0 + kh) * ROW + kw)
                                    wlen = nrows * ROW
                                    rhs = xs_sb[:, q : q + wlen].rearrange(
                                        "p (r c) -> p r c", c=ROW
                                    )[:, :, cw0:cw1]
                                    nc.tensor.matmul(
                                        pt[:], wt_sb[:, _tap_idx(kd, kh, kw), :],
                                        rhs,
                                        start=(i == 0), stop=(i == n_mm - 1),
                                    )
                                    i += 1
                        # ut = psum*sqrt(2) + noise_pre (pre-scaled on host)
                        uv = ut3[:, r0:r1, cw0:cw1]
                        nc.vector.scalar_tensor_tensor(
                            uv, pt[:], LRELU_GAIN,
                            nz3[:, r0:r1, cw0:cw1],
                            mybir.AluOpType.mult, mybir.AluOpType.add,
                        )
                        if last and r0 == 15:
                            # all-DVE post for the very last sub: no
                            # cross-engine hops on the drain path.
                            # lrelu(z) == max(z, 0.2*z)
                            nc.vector.tensor_scalar_add(uv, uv, bb_sb[:, 0:1])
                            nc.vector.scalar_tensor_tensor(
                                uv, uv, LRELU_ALPHA, uv,
                                mybir.AluOpType.mult, mybir.AluOpType.max,
                            )
                        else:
                            nc.scalar.activation(
                                uv, uv, AF.Prelu,
                                bias=bb_sb[:, 0:1], scale=1.0,
                                alpha=LRELU_ALPHA,
                            )
                        nc.vector.tensor_scalar(
                            uv, uv, CLAMP, -CLAMP,
                            mybir.AluOpType.min, mybir.AluOpType.max,
                        )
                        if last and r1 <= 12:
                            nc.sync.dma_start(
                                y[:, off + r0 * RES : off + r1 * RES],
                                ut[:, r0 * RES : r1 * RES],
                            )
                    if last:
                        # one DMA for the final two subs (rows 12..16)
                        nc.sync.dma_start(
                            y[:, off + 384 : off + NCHUNK], ut[:, 384:NCHUNK]
                        )
                    else:
                        nc.sync.dma_start(y[:, off : off + NCHUNK], ut[:])

    nc.compile()
    return nc


def _get_nc():
    global _NC_CACHE
    if _NC_CACHE is None:
        _NC_CACHE = build_nc()
    return _NC_CACHE


def _make_core_inputs(x, w, affine_weight, affine_bias, weight, noise_const,
                      noise_strength, bias):
    """Host-side prep: styles, modulation, demodulation, layout, bf16."""
    styles = (w @ affine_weight.T) / math.sqrt(W_DIM) + affine_bias      # [B,Cin]
    wmod = weight[None] * styles[:, None, :, None, None, None]           # [B,Co,Ci,3,3,3]
    dco = 1.0 / np.sqrt((wmod ** 2).sum(axis=(2, 3, 4, 5)) + 1e-8)       # [B,Co]
    wd = wmod * dco[:, :, None, None, None, None]                        # demodulated

    xb = x.astype(BF16)                                                  # [B,Ci,32,32,32]
    nzs = (noise_const * float(noise_strength.reshape(-1)[0])
           * LRELU_GAIN).astype(BF16)                                    # [32,32,32]
    bgain = bias * LRELU_GAIN
    bb_host = np.stack(
        [bgain,                                  # prelu bias
         -CLAMP / LRELU_ALPHA - bgain,           # pre-act clamp lo
         CLAMP - bgain],                         # pre-act clamp hi
        axis=1,
    ).astype(np.float32)                                                 # [P,3]

    in_maps = []
    for c in range(8):
        b, half = divmod(c, 2)
        # weights: [ci, kd, kh, kw, co]; kd reversed for the upper half
        arr = wd[b].transpose(1, 2, 3, 4, 0)
        if half == 1:
            arr = arr[:, ::-1]
        wt_host = np.ascontiguousarray(arr.reshape(P, 27, P)).astype(BF16)

        # slab: local slice j = global (j) or (31-j); 17 slices incl. halo
        if half == 0:
            src = xb[b, :, 0:NSLICES]                      # 0..16
            nz_loc = nzs[0:DHALF]
        else:
            src = xb[b, :, 31:31 - NSLICES:-1]             # 31..15
            nz_loc = nzs[31:31 - DHALF:-1]
        slab = np.zeros((P, NSLAB), BF16)
        view = slab[:, LEAD:LEAD + NSLICES * SLICE].reshape(P, NSLICES, ROW, ROW)
        view[:, :, :RES, :RES] = src
        nz_host = np.ascontiguousarray(nz_loc.reshape(1, NOUT))

        in_maps.append({
            "xs": slab,
            "wt": wt_host,
            "nz": nz_host,
            "bb": bb_host,
        })
    return in_maps


def kernel(x, w, affine_weight, affine_bias, weight, noise_const,
           noise_strength, bias):
    global LAST_EXEC_NS
    x = np.asarray(x, np.float32)
    w = np.asarray(w, np.float32)
    affine_weight = np.asarray(affine_weight, np.float32)
    affine_bias = np.asarray(affine_bias, np.float32)
    weight = np.asarray(weight, np.float32)
    noise_const = np.asarray(noise_const, np.float32)
    noise_strength = np.asarray(noise_strength, np.float32)
    bias = np.asarray(bias, np.float32)

    nc = _get_nc()
    in_maps = _make_core_inputs(
        x, w, affine_weight, affine_bias, weight, noise_const,
        noise_strength, bias,
    )
    trace = bool(os.environ.get("KERNEL_TRACE"))
    if trace:
        from concourse.bass_utils import axon_active

        if axon_active():
            try:  # axon NTFF capture needs the profile hook; absent in some pods
                from antenv.axon_hooks import get_axon_ntff_profile_hook  # noqa: F401
            except ImportError:
                trace = False
    res = run_bass_kernel_spmd(nc, in_maps, core_ids=list(range(8)), trace=trace)
    LAST_EXEC_NS = res.exec_time_ns

    out = np.empty((B, P, RES, RES, RES), np.float32)
    for c in range(8):
        b, half = divmod(c, 2)
        yc = res.results[c]["y"].reshape(P, DHALF, RES, RES)
        if half == 0:
            out[b, :, 0:DHALF] = yc
        else:
            out[b, :, 31:31 - DHALF:-1] = yc
    return out


# revision 33
# speedup vs baseline: 1.0174x; 1.0002x over previous
"""TRN2 Bass kernel for nn_SynthesisLayer (StyleGAN-style modulated 3D conv).

Math: for each sample b
  styles = w[b] @ affine_weight.T / sqrt(512) + affine_bias          [Cin]
  wmod   = weight * styles[None,:,None]                              [Co,Ci,27]
  dcoef  = rsqrt(sum_{ci,k} wmod^2 + 1e-8)                           [Co]
  y      = dcoef * conv3d(x[b], wmod, pad=1) + noise_const*ns + bias
  out    = clip(lrelu(y)*sqrt(2), -256, 256)

Device implementation (per core):
  - styles/modulation/demodulation are folded on the host: the device
    receives pre-modulated, pre-demodulated bf16 weights, so the conv is
    27 shifted bf16 matmuls (contraction over Cin=128 on partitions)
    accumulated in PSUM, then scale+noise+prelu+clamp on DVE/ACT.
  - Sharding: 8 cores = 4 samples x 2 D-halves. The upper D-half runs
    D-REVERSED (reversed input slices, kd-reversed weights, output
    un-reversed on host) so that for EVERY core the first output slice
    is a volume boundary: its 9 kd=-1 taps are identically zero and are
    skipped. Likewise output rows 0/31 are split into 1-row sub-chunks
    that skip the zero-row kh taps (d>=1 only: the d=0 instruction
    stream is kept in the exact shape that starts the PE past its
    p-state ramp window). Together ~4% of PE work is skipped.
  - x slab (17 slices + guards, whole D-half) is SBUF-resident in bf16
    and DMA-streamed slice-by-slice so matmuls chase the DMA front;
    weights/noise also ship bf16 (halves DMA); PSUM stays f32.
  - The final chunk is split into small sub-chunks with an all-DVE
    post chain and a merged last output DMA to shorten the drain tail.
"""

import math
import os
import sys

for _p in ("/opt/trn_rl_repo", "/root/.axon_site/_ro/trn_rl_repo"):
    if os.path.isdir(_p) and _p not in sys.path:
        sys.path.insert(0, _p)

import numpy as np
import ml_dtypes

import concourse.mybir as mybir
from concourse import bacc
from concourse.tile import TileContext
from concourse.bass_utils import run_bass_kernel_spmd

BF16 = ml_dtypes.bfloat16

P = 128          # Cin = Cout = 128
RES = 32
B = 4
W_DIM = 512
ROW = 33                   # padded row width  (32 real + 1 zero)
SLICE = ROW * ROW          # 1089: padded slice (32 real rows + 1 zero row)
LEAD = 34                  # leading zero guard (one row + one elem)
NSLICES = 17               # 16 output slices + 1 far-side halo
DHALF = 16                 # output D slices per core
NOUT = DHALF * RES * RES   # 16384
NSLAB = LEAD + NSLICES * SLICE + 46   # 18593; max AP end = 18548
NCHUNK = 512               # psum tile free size (one PSUM bank of fp32)
LRELU_ALPHA = 0.2
LRELU_GAIN = math.sqrt(2.0)
CLAMP = 256.0

f32 = mybir.dt.float32
bf16 = mybir.dt.bfloat16
AF = mybir.ActivationFunctionType

_NC_CACHE = None
LAST_EXEC_NS = None

# tap index in the weight tensor: k = (kd+1)*9 + (kh+1)*3 + (kw+1)
def _tap_idx(kd, kh, kw):
    return (kd + 1) * 9 + (kh + 1) * 3 + (kw + 1)


def build_nc():
    nc = bacc.Bacc("TRN2", target_bir_lowering=False, debug=False, num_devices=8)

    xs = nc.dram_tensor("xs", [P, NSLAB], bf16, kind="ExternalInput")
    wt = nc.dram_tensor("wt", [P, 27, P], bf16, kind="ExternalInput")
    nz = nc.dram_tensor("nz", [1, NOUT], bf16, kind="ExternalInput")
    bb = nc.dram_tensor("bb", [P, 3], f32, kind="ExternalInput")  # bias', lo, hi
    y = nc.dram_tensor("y", [P, NOUT], f32, kind="ExternalOutput")

    with TileContext(nc) as tc:
        with (
            tc.tile_pool(name="slab", bufs=1) as slabp,
            tc.tile_pool(name="nzp", bufs=4) as nzp,
            tc.tile_pool(name="outp", bufs=4) as outp,
            tc.tile_pool(name="cpsum", bufs=8, space="PSUM") as cpsum,
        ):
            xs_sb = slabp.tile([P, NSLAB], bf16)
            wt_sb = slabp.tile([P, 27, P], bf16)
            bb_sb = slabp.tile([P, 3], f32)

            # p-state warm-up: start the PE's continuous-busy clock early on
            # zeroed scratch so the first real matmuls run at full speed
            sc = slabp.tile([P, NCHUNK], bf16)
            nc.any.memset(sc[:], 0)
            for _ in range(4):
                wps = cpsum.tile([P, NCHUNK], f32, tag="conv")
                nc.tensor.matmul(wps[:], sc[:, :P], sc[:], start=True, stop=True)

            def dma_slice(j):
                """DMA padded slice j (first includes LEAD, last the tail)."""
                lo = 0 if j == 0 else LEAD + j * SLICE
                hi = NSLAB if j == NSLICES - 1 else LEAD + (j + 1) * SLICE
                nc.sync.dma_start(xs_sb[:, lo:hi], xs[:, lo:hi])

            # startup order: tiny first pieces so the first matmul's inputs
            # land as early as possible (descriptor issue is ~650ns each)
            nc.sync.dma_start(xs_sb[:, 0:600], xs[:, 0:600])       # s0 rows<=16
            nc.sync.dma_start(wt_sb[:, 9:12, :], wt[:, 9:12, :])   # kd=0,kh=-1
            nc.sync.dma_start(wt_sb[:, 12:18, :], wt[:, 12:18, :])  # kd=0 rest
            nc.sync.dma_start(xs_sb[:, 600 : LEAD + SLICE],
                              xs[:, 600 : LEAD + SLICE])            # s0 rest
            nc.sync.dma_start(wt_sb[:, 18:27, :], wt[:, 18:27, :])  # kd=+1
            dma_slice(1)
            nc.sync.dma_start(wt_sb[:, 0:9, :], wt[:, 0:9, :])      # kd=-1
            nc.sync.dma_start(bb_sb[:], bb[:])
            dma_slice(2)
            dma_slice(3)

            next_slice = 4

            for d in range(DHALF):
                if next_slice < NSLICES:
                    dma_slice(next_slice)
                    next_slice += 1
                kds = (0, 1) if d == 0 else (-1, 0, 1)
                KH_ALL = (-1, 0, 1)
                KW_ALL = (-1, 0, 1)
                for h in range(2):
                    # boundary subs skip zero-guard taps: row 0 has no kh=-1
                    # contribution, row 31 no kh=+1; likewise col 0 skips
                    # kw=-1 and col 31 kw=+1 via 1-col subs. d=0 keeps the
                    # exact stream shape that keeps the PE p-state warm; the
                    # final chunk keeps row-splits so the drain tail is short.
                    last = d == DHALF - 1 and h == 1
                    if d < 1:
                        subs = [(0, 16, 0, 32, KH_ALL, KW_ALL)]
                    elif h == 0:
                        subs = [(0, 1, 0, 32, (0, 1), KW_ALL),
                                (1, 16, 0, 1, KH_ALL, (0, 1)),
                                (1, 16, 1, 31, KH_ALL, KW_ALL),
                                (1, 16, 31, 32, KH_ALL, (-1, 0))]
                    elif last:
                        subs = [(0, 4, 0, 32, KH_ALL, KW_ALL),
                                (4, 8, 0, 32, KH_ALL, KW_ALL),
                                (8, 12, 0, 32, KH_ALL, KW_ALL),
                                (12, 15, 0, 32, KH_ALL, KW_ALL),
                                (15, 16, 0, 32, (-1, 0), KW_ALL)]
                    else:
                        subs = [(0, 15, 0, 1, KH_ALL, (0, 1)),
                                (0, 15, 1, 31, KH_ALL, KW_ALL),
                                (0, 15, 31, 32, KH_ALL, (-1, 0)),
                                (15, 16, 0, 32, (-1, 0), KW_ALL)]
                    off = d * 1024 + h * NCHUNK
                    nz_bc = nzp.tile([P, 1, NCHUNK], bf16, tag="nz")
                    nc.sync.dma_start(
                        nz_bc[:],
                        nz[:, off : off + NCHUNK].partition_broadcast(P),
                    )
                    ut = outp.tile([P, NCHUNK], f32, tag="out")
                    ut3 = ut[:].rearrange("p (r c) -> p r c", c=RES)
                    nz3 = nz_bc[:, 0, :].rearrange("p (r c) -> p r c", c=RES)
                    for r0, r1, cw0, cw1, khs, kws in subs:
                        nrows = r1 - r0
                        ncw = cw1 - cw0
                        pt = cpsum.tile([P, nrows, ncw], f32, tag="conv")
                        n_mm = len(kds) * len(khs) * len(kws)
                        i = 0
                        for kd in kds:
                            for kh in khs:
                                for kw in kws:
                                    q = (LEAD + (d + kd) * SLICE
                                         + (16 * h + r0 + kh) * ROW + kw)
                                    wlen = nrows * ROW
                                    rhs = xs_sb[:, q : q + wlen].rearrange(
                                        "p (r c) -> p r c", c=ROW
                                    )[:, :, cw0:cw1]
                                    nc.tensor.matmul(
                                        pt[:], wt_sb[:, _tap_idx(kd, kh, kw), :],
                                        rhs,
                                        start=(i == 0), stop=(i == n_mm - 1),
                                    )
                                    i += 1
                        # ut = psum*sqrt(2) + noise_pre (pre-scaled on host)
                        uv = ut3[:, r0:r1, cw0:cw1]
                        nc.vector.scalar_tensor_tensor(
                            uv, pt[:], LRELU_GAIN,
                            nz3[:, r0:r1, cw0:cw1],
                            mybir.AluOpType.mult, mybir.AluOpType.add,
                        )
                        if last and r0 == 15:
                            # all-DVE post for the very last sub: no
                            # cross-engine hops on the drain path.
                            # lrelu(z) == max(z, 0.2*z)
                            nc.vector.tensor_scalar_add(uv, uv, bb_sb[:, 0:1])
                            nc.vector.scalar_tensor_tensor(
                                uv, uv, LRELU_ALPHA, uv,
                                mybir.AluOpType.mult, mybir.AluOpType.max,
                            )
                        else:
                            nc.scalar.activation(
                                uv, uv, AF.Prelu,
                                bias=bb_sb[:, 0:1], scale=1.0,
                                alpha=LRELU_ALPHA,
                            )
                        nc.vector.tensor_scalar(
                            uv, uv, CLAMP, -CLAMP,
                            mybir.AluOpType.min, mybir.AluOpType.max,
                        )
                        if last and r1 <= 12:
                            nc.sync.dma_start(
                                y[:, off + r0 * RES : off + r1 * RES],
                                ut[:, r0 * RES : r1 * RES],
                            )
                    if last:
                        # one DMA for the final two subs (rows 12..16)
                        nc.sync.dma_start(
                            y[:, off + 384 : off + NCHUNK], ut[:, 384:NCHUNK]
                        )
                    else:
                        nc.sync.dma_start(y[:, off : off + NCHUNK], ut[:])

    nc.compile()
    return nc


def _get_nc():
    global _NC_CACHE
    if _NC_CACHE is None:
        _NC_CACHE = build_nc()
    return _NC_CACHE


def _make_core_inputs(x, w, affine_weight, affine_bias, weight, noise_const,
                      noise_strength, bias):
    """Host-side prep: styles, modulation, demodulation, layout, bf16."""
    styles = (w @ affine_weight.T) / math.sqrt(W_DIM) + affine_bias      # [B,Cin]
    wmod = weight[None] * styles[:, None, :, None, None, None]           # [B,Co,Ci,3,3,3]
    dco = 1.0 / np.sqrt((wmod ** 2).sum(axis=(2, 3, 4, 5)) + 1e-8)       # [B,Co]
    wd = wmod * dco[:, :, None, None, None, None]                        # demodulated

    xb = x.astype(BF16)                                                  # [B,Ci,32,32,32]
    nzs = (noise_const * float(noise_strength.reshape(-1)[0])
           * LRELU_GAIN).astype(BF16)                                    # [32,32,32]
    bgain = bias * LRELU_GAIN
    bb_host = np.stack(
        [bgain,                                  # prelu bias
         -CLAMP / LRELU_ALPHA - bgain,           # pre-act clamp lo
         CLAMP - bgain],                         # pre-act clamp hi
        axis=1,
    ).astype(np.float32)                                                 # [P,3]

    in_maps = []
    for c in range(8):
        b, half = divmod(c, 2)
        # weights: [ci, kd, kh, kw, co]; kd reversed for the upper half
        arr = wd[b].transpose(1, 2, 3, 4, 0)
        if half == 1:
            arr = arr[:, ::-1]
        wt_host = np.ascontiguousarray(arr.reshape(P, 27, P)).astype(BF16)

        # slab: local slice j = global (j) or (31-j); 17 slices incl. halo
        if half == 0:
            src = xb[b, :, 0:NSLICES]                      # 0..16
            nz_loc = nzs[0:DHALF]
        else:
            src = xb[b, :, 31:31 - NSLICES:-1]             # 31..15
            nz_loc = nzs[31:31 - DHALF:-1]
        slab = np.zeros((P, NSLAB), BF16)
        view = slab[:, LEAD:LEAD + NSLICES * SLICE].reshape(P, NSLICES, ROW, ROW)
        view[:, :, :RES, :RES] = src
        nz_host = np.ascontiguousarray(nz_loc.reshape(1, NOUT))

        in_maps.append({
            "xs": slab,
            "wt": wt_host,
            "nz": nz_host,
            "bb": bb_host,
        })
    return in_maps


def kernel(x, w, affine_weight, affine_bias, weight, noise_const,
           noise_strength, bias):
    global LAST_EXEC_NS
    x = np.asarray(x, np.float32)
    w = np.asarray(w, np.float32)
    affine_weight = np.asarray(affine_weight, np.float32)
    affine_bias = np.asarray(affine_bias, np.float32)
    weight = np.asarray(weight, np.float32)
    noise_const = np.asarray(noise_const, np.float32)
    noise_strength = np.asarray(noise_strength, np.float32)
    bias = np.asarray(bias, np.float32)

    nc = _get_nc()
    in_maps = _make_core_inputs(
        x, w, affine_weight, affine_bias, weight, noise_const,
        noise_strength, bias,
    )
    trace = bool(os.environ.get("KERNEL_TRACE"))
    if trace:
        from concourse.bass_utils import axon_active

        if axon_active():
            try:  # axon NTFF capture needs the profile hook; absent in some pods
                from antenv.axon_hooks import get_axon_ntff_profile_hook  # noqa: F401
            except ImportError:
                trace = False
    res = run_bass_kernel_spmd(nc, in_maps, core_ids=list(range(8)), trace=trace)
    LAST_EXEC_NS = res.exec_time_ns

    out = np.empty((B, P, RES, RES, RES), np.float32)
    for c in range(8):
        b, half = divmod(c, 2)
        yc = res.results[c]["y"].reshape(P, DHALF, RES, RES)
        if half == 0:
            out[b, :, 0:DHALF] = yc
        else:
            out[b, :, 31:31 - DHALF:-1] = yc
    return out


# revision 40
# speedup vs baseline: 1.0191x; 1.0017x over previous
"""TRN2 Bass kernel for nn_SynthesisLayer (StyleGAN-style modulated 3D conv).

Math: for each sample b
  styles = w[b] @ affine_weight.T / sqrt(512) + affine_bias          [Cin]
  wmod   = weight * styles[None,:,None]                              [Co,Ci,27]
  dcoef  = rsqrt(sum_{ci,k} wmod^2 + 1e-8)                           [Co]
  y      = dcoef * conv3d(x[b], wmod, pad=1) + noise_const*ns + bias
  out    = clip(lrelu(y)*sqrt(2), -256, 256)

Device implementation (per core):
  - styles/modulation/demodulation are folded on the host: the device
    receives pre-modulated, pre-demodulated bf16 weights, so the conv is
    27 shifted bf16 matmuls (contraction over Cin=128 on partitions)
    accumulated in PSUM, then scale+noise+prelu+clamp on DVE/ACT.
  - Sharding: 8 cores = 4 samples x 2 D-halves. The upper D-half runs
    D-REVERSED (reversed input slices, kd-reversed weights, output
    un-reversed on host) so that for EVERY core the first output slice
    is a volume boundary: its 9 kd=-1 taps are identically zero and are
    skipped. Likewise output rows 0/31 are split into 1-row sub-chunks
    that skip the zero-row kh taps (d>=1 only: the d=0 instruction
    stream is kept in the exact shape that starts the PE past its
    p-state ramp window). Together ~4% of PE work is skipped.
  - x slab (17 slices + guards, whole D-half) is SBUF-resident in bf16
    and DMA-streamed slice-by-slice so matmuls chase the DMA front;
    weights/noise also ship bf16 (halves DMA); PSUM stays f32.
  - The final chunk is split into small sub-chunks with an all-DVE
    post chain and a merged last output DMA to shorten the drain tail.
"""

import math
import os
import sys

for _p in ("/opt/trn_rl_repo", "/root/.axon_site/_ro/trn_rl_repo"):
    if os.path.isdir(_p) and _p not in sys.path:
        sys.path.insert(0, _p)

import numpy as np
import ml_dtypes

import concourse.mybir as mybir
from concourse import bacc
from concourse.tile import TileContext
from concourse.bass_utils import run_bass_kernel_spmd

BF16 = ml_dtypes.bfloat16

P = 128          # Cin = Cout = 128
RES = 32
B = 4
W_DIM = 512
ROW = 33                   # padded row width  (32 real + 1 zero)
SLICE = ROW * ROW          # 1089: padded slice (32 real rows + 1 zero row)
LEAD = 34                  # leading zero guard (one row + one elem)
NSLICES = 17               # 16 output slices + 1 far-side halo
DHALF = 16                 # output D slices per core
NOUT = DHALF * RES * RES   # 16384
NSLAB = LEAD + NSLICES * SLICE + 46   # 18593; max AP end = 18548
NCHUNK = 512               # psum tile free size (one PSUM bank of fp32)
LRELU_ALPHA = 0.2
LRELU_GAIN = math.sqrt(2.0)
CLAMP = 256.0

f32 = mybir.dt.float32
bf16 = mybir.dt.bfloat16
AF = mybir.ActivationFunctionType

_NC_CACHE = None
LAST_EXEC_NS = None

# tap index in the weight tensor: k = (kd+1)*9 + (kh+1)*3 + (kw+1)
def _tap_idx(kd, kh, kw):
    return (kd + 1) * 9 + (kh + 1) * 3 + (kw + 1)


def build_nc():
    nc = bacc.Bacc("TRN2", target_bir_lowering=False, debug=False, num_devices=8)

    xs = nc.dram_tensor("xs", [P, NSLAB], bf16, kind="ExternalInput")
    wt = nc.dram_tensor("wt", [P, 27, P], bf16, kind="ExternalInput")
    nz = nc.dram_tensor("nz", [1, NOUT], bf16, kind="ExternalInput")
    bb = nc.dram_tensor("bb", [P, 3], f32, kind="ExternalInput")  # bias', lo, hi
    y = nc.dram_tensor("y", [P, NOUT], f32, kind="ExternalOutput")

    with TileContext(nc) as tc:
        with (
            tc.tile_pool(name="slab", bufs=1) as slabp,
            tc.tile_pool(name="nzp", bufs=4) as nzp,
            tc.tile_pool(name="outp", bufs=4) as outp,
            tc.tile_pool(name="cpsum", bufs=8, space="PSUM") as cpsum,
        ):
            xs_sb = slabp.tile([P, NSLAB], bf16)
            wt_sb = slabp.tile([P, 27, P], bf16)
            bb_sb = slabp.tile([P, 3], f32)

            # p-state warm-up: start the PE's continuous-busy clock early on
            # zeroed scratch so the first real matmuls run at full speed
            sc = slabp.tile([P, NCHUNK], bf16)
            nc.any.memset(sc[:], 0)
            for _ in range(4):
                wps = cpsum.tile([P, NCHUNK], f32, tag="conv")
                nc.tensor.matmul(wps[:], sc[:, :P], sc[:], start=True, stop=True)

            def dma_slice(j):
                """DMA padded slice j (first includes LEAD, last the tail)."""
                lo = 0 if j == 0 else LEAD + j * SLICE
                hi = NSLAB if j == NSLICES - 1 else LEAD + (j + 1) * SLICE
                nc.sync.dma_start(xs_sb[:, lo:hi], xs[:, lo:hi])

            # startup order: tiny first pieces so the first matmul's inputs
            # land as early as possible (descriptor issue is ~650ns each)
            nc.sync.dma_start(xs_sb[:, 0:600], xs[:, 0:600])       # s0 rows<=16
            nc.sync.dma_start(wt_sb[:, 9:12, :], wt[:, 9:12, :])   # kd=0,kh=-1
            nc.sync.dma_start(wt_sb[:, 12:18, :], wt[:, 12:18, :])  # kd=0 rest
            nc.sync.dma_start(xs_sb[:, 600 : LEAD + SLICE],
                              xs[:, 600 : LEAD + SLICE])            # s0 rest
            nc.sync.dma_start(wt_sb[:, 18:27, :], wt[:, 18:27, :])  # kd=+1
            dma_slice(1)
            nc.sync.dma_start(wt_sb[:, 0:9, :], wt[:, 0:9, :])      # kd=-1
            nc.sync.dma_start(bb_sb[:], bb[:])
            dma_slice(2)
            dma_slice(3)

            next_slice = 4

            for d in range(DHALF):
                if next_slice < NSLICES:
                    dma_slice(next_slice)
                    next_slice += 1
                kds = (0, 1) if d == 0 else (-1, 0, 1)
                KH_ALL = (-1, 0, 1)
                KW_ALL = (-1, 0, 1)
                for h in range(2):
                    # boundary subs skip zero-guard taps: row 0 has no kh=-1
                    # contribution, row 31 no kh=+1; likewise col 0 skips
                    # kw=-1 and col 31 kw=+1 via 1-col subs. d=0 keeps the
                    # exact stream shape that keeps the PE p-state warm; the
                    # final chunk keeps row-splits so the drain tail is short.
                    last = d == DHALF - 1 and h == 1
                    if h == 0:
                        subs = [(1, 16, 1, 31, KH_ALL, KW_ALL),
                                (0, 1, 0, 32, (0, 1), KW_ALL),
                                (1, 16, 0, 1, KH_ALL, (0, 1)),
                                (1, 16, 31, 32, KH_ALL, (-1, 0))]
                    elif last:
                        subs = [(0, 4, 0, 32, KH_ALL, KW_ALL),
                                (4, 8, 0, 32, KH_ALL, KW_ALL),
                                (8, 12, 0, 32, KH_ALL, KW_ALL),
                                (12, 15, 0, 32, KH_ALL, KW_ALL),
                                (15, 16, 0, 32, (-1, 0), KW_ALL)]
                    else:
                        subs = [(0, 15, 1, 31, KH_ALL, KW_ALL),
                                (0, 15, 0, 1, KH_ALL, (0, 1)),
                                (0, 15, 31, 32, KH_ALL, (-1, 0)),
                                (15, 16, 0, 32, (-1, 0), KW_ALL)]
                    off = d * 1024 + h * NCHUNK
                    nz_bc = nzp.tile([P, 1, NCHUNK], bf16, tag="nz")
                    nc.sync.dma_start(
                        nz_bc[:],
                        nz[:, off : off + NCHUNK].partition_broadcast(P),
                    )
                    ut = outp.tile([P, NCHUNK], f32, tag="out")
                    ut3 = ut[:].rearrange("p (r c) -> p r c", c=RES)
                    nz3 = nz_bc[:, 0, :].rearrange("p (r c) -> p r c", c=RES)
                    for r0, r1, cw0, cw1, khs, kws in subs:
                        nrows = r1 - r0
                        ncw = cw1 - cw0
                        pt = cpsum.tile([P, nrows, ncw], f32, tag="conv")
                        n_mm = len(kds) * len(khs) * len(kws)
                        i = 0
                        for kd in kds:
                            for kh in khs:
                                for kw in kws:
                                    q = (LEAD + (d + kd) * SLICE
                                         + (16 * h + r0 + kh) * ROW + kw)
                                    wlen = nrows * ROW
                                    rhs = xs_sb[:, q : q + wlen].rearrange(
                                        "p (r c) -> p r c", c=ROW
                                    )[:, :, cw0:cw1]
                                    nc.tensor.matmul(
                                        pt[:], wt_sb[:, _tap_idx(kd, kh, kw), :],
                                        rhs,
                                        start=(i == 0), stop=(i == n_mm - 1),
                                    )
                                    i += 1
                        # ut = psum*sqrt(2) + noise_pre (pre-scaled on host)
                        uv = ut3[:, r0:r1, cw0:cw1]
                        nc.vector.scalar_tensor_tensor(
                            uv, pt[:], LRELU_GAIN,
                            nz3[:, r0:r1, cw0:cw1],
                            mybir.AluOpType.mult, mybir.AluOpType.add,
                        )
                        if last and r0 == 15:
                            # all-DVE post for the very last sub: no
                            # cross-engine hops on the drain path.
                            # lrelu(z) == max(z, 0.2*z)
                            nc.vector.tensor_scalar_add(uv, uv, bb_sb[:, 0:1])
                            nc.vector.scalar_tensor_tensor(
                                uv, uv, LRELU_ALPHA, uv,
                                mybir.AluOpType.mult, mybir.AluOpType.max,
                            )
                        else:
                            nc.scalar.activation(
                                uv, uv, AF.Prelu,
                                bias=bb_sb[:, 0:1], scale=1.0,
                                alpha=LRELU_ALPHA,
                            )
                        nc.vector.tensor_scalar(
                            uv, uv, CLAMP, -CLAMP,
                            mybir.AluOpType.min, mybir.AluOpType.max,
                        )
                        if last and r1 <= 12:
                            nc.sync.dma_start(
                                y[:, off + r0 * RES : off + r1 * RES],
                                ut[:, r0 * RES : r1 * RES],
                            )
                    if last:
                        # one DMA for the final two subs (rows 12..16)
                        nc.sync.dma_start(
                            y[:, off + 384 : off + NCHUNK], ut[:, 384:NCHUNK]
                        )
                    else:
                        nc.sync.dma_start(y[:, off : off + NCHUNK], ut[:])

    nc.compile()
    return nc


def _get_nc():
    global _NC_CACHE
    if _NC_CACHE is None:
        _NC_CACHE = build_nc()
    return _NC_CACHE


def _make_core_inputs(x, w, affine_weight, affine_bias, weight, noise_const,
                      noise_strength, bias):
    """Host-side prep: styles, modulation, demodulation, layout, bf16."""
    styles = (w @ affine_weight.T) / math.sqrt(W_DIM) + affine_bias      # [B,Cin]
    wmod = weight[None] * styles[:, None, :, None, None, None]           # [B,Co,Ci,3,3,3]
    dco = 1.0 / np.sqrt((wmod ** 2).sum(axis=(2, 3, 4, 5)) + 1e-8)       # [B,Co]
    wd = wmod * dco[:, :, None, None, None, None]                        # demodulated

    xb = x.astype(BF16)                                                  # [B,Ci,32,32,32]
    nzs = (noise_const * float(noise_strength.reshape(-1)[0])
           * LRELU_GAIN).astype(BF16)                                    # [32,32,32]
    bgain = bias * LRELU_GAIN
    bb_host = np.stack(
        [bgain,                                  # prelu bias
         -CLAMP / LRELU_ALPHA - bgain,           # pre-act clamp lo
         CLAMP - bgain],                         # pre-act clamp hi
        axis=1,
    ).astype(np.float32)                                                 # [P,3]

    in_maps = []
    for c in range(8):
        b, half = divmod(c, 2)
        # weights: [ci, kd, kh, kw, co]; kd reversed for the upper half
        arr = wd[b].transpose(1, 2, 3, 4, 0)
        if half == 1:
            arr = arr[:, ::-1]
        wt_host = np.ascontiguousarray(arr.reshape(P, 27, P)).astype(BF16)

        # slab: local slice j = global (j) or (31-j); 17 slices incl. halo
        if half == 0:
            src = xb[b, :, 0:NSLICES]                      # 0..16
            nz_loc = nzs[0:DHALF]
        else:
            src = xb[b, :, 31:31 - NSLICES:-1]             # 31..15
            nz_loc = nzs[31:31 - DHALF:-1]
        slab = np.zeros((P, NSLAB), BF16)
        view = slab[:, LEAD:LEAD + NSLICES * SLICE].reshape(P, NSLICES, ROW, ROW)
        view[:, :, :RES, :RES] = src
        nz_host = np.ascontiguousarray(nz_loc.reshape(1, NOUT))

        in_maps.append({
            "xs": slab,
            "wt": wt_host,
            "nz": nz_host,
            "bb": bb_host,
        })
    return in_maps


def kernel(x, w, affine_weight, affine_bias, weight, noise_const,
           noise_strength, bias):
    global LAST_EXEC_NS
    x = np.asarray(x, np.float32)
    w = np.asarray(w, np.float32)
    affine_weight = np.asarray(affine_weight, np.float32)
    affine_bias = np.asarray(affine_bias, np.float32)
    weight = np.asarray(weight, np.float32)
    noise_const = np.asarray(noise_const, np.float32)
    noise_strength = np.asarray(noise_strength, np.float32)
    bias = np.asarray(bias, np.float32)

    nc = _get_nc()
    in_maps = _make_core_inputs(
        x, w, affine_weight, affine_bias, weight, noise_const,
        noise_strength, bias,
    )
    trace = bool(os.environ.get("KERNEL_TRACE"))
    if trace:
        from concourse.bass_utils import axon_active

        if axon_active():
            try:  # axon NTFF capture needs the profile hook; absent in some pods
                from antenv.axon_hooks import get_axon_ntff_profile_hook  # noqa: F401
            except ImportError:
                trace = False
    res = run_bass_kernel_spmd(nc, in_maps, core_ids=list(range(8)), trace=trace)
    LAST_EXEC_NS = res.exec_time_ns

    out = np.empty((B, P, RES, RES, RES), np.float32)
    for c in range(8):
        b, half = divmod(c, 2)
        yc = res.results[c]["y"].reshape(P, DHALF, RES, RES)
        if half == 0:
            out[b, :, 0:DHALF] = yc
        else:
            out[b, :, 31:31 - DHALF:-1] = yc
    return out
